# revision 1
# baseline (speedup 1.0000x reference)
"""DocumentEncoder kernel for Trainium2 (8 NeuronCores, Bass/Tile).

Reference computation (B=256, L=512, D=1024, V=50000):
    emb    = emb_table[tokens + 1]            # [B, L, D] gather
    hidden = emb.mean(axis=1)                 # [B, D]
    ha     = einsum('bld,de->ble', emb, W_b)
    scores = einsum('ble,be->bl', ha, hidden)
    attn   = softmax(scores, axis=1)
    ct     = einsum('bl,bld->bd', attn, emb)  # [B, D]

Key algebraic rewrite: scores[b,l] = emb[b,l,:] . (W_b @ hidden[b]) -- the
[B,L,D]x[D,D] einsum collapses to a [B,D]x[D,D] one (~230x less compute),
leaving the embedding-row gather as the dominant cost.

Sharding: data-parallel over B (32 docs per core), table/W_b replicated.

The device-side gather uses the DMAGatherAnt SWDGE ucode, whose indices are
int16: each core therefore gets a host-built compact table holding only the
(<= 16384, so int16-safe) embedding rows its own docs reference, with token
ids remapped accordingly.

Per-core device program, groups of G=4 docs:
  1. dma_gather per 128-token block: 128 rows -> E [128p, 1, 1024d]
     (token l at partition l%128); 30 block-sized SBUF slots give the
     gather a deep prefetch window and fine-grained slot recycling
  2. s_j = sum_l emb_j on PE as a [1, 1024] row (fp32r, ones stationary,
     E moving)
  3. PE-transpose of the 4 rows into s^T columns of one [128, 32] tile;
     batched V = (W_b^T/L)^T s contraction for the 4 docs (stride-8
     stationary)
  4. per doc, streamed per 128-token block: PE one-hot broadcast of v_j
     across partitions (both halves up front); then for each block c: DVE
     fused multiply+accumulate for scores_c, ACT exp with fused
     per-partition Z accumulation, and the block's term of the
     unnormalized context sum on PE (exp stationary, E moving) -- so each
     E block is released as soon as its term is in. Finally a tiny PE
     ones-matmul for the total Z, DVE reciprocal, ACT scale by 1/Z, and
     one 4KB DMA out per doc.

All PE matmuls run in float32r (fp32 bits, reduced-precision PE mode, 1
cycle/row at N>=256 vs 4 for full fp32); the end-to-end error vs the fp32
reference is ~1e-3 of the output scale.
"""

from contextlib import ExitStack

import numpy as np

import concourse.tile as tile
from concourse import bacc, mybir
from concourse.bass_utils import run_bass_kernel_spmd
from concourse.masks import make_identity

B, L, D, V = 256, 512, 1024, 50000
N_CORES = 8
DOCS = B // N_CORES          # 32 docs per core
G = 4                        # docs per group (col-tiling width)
NG = DOCS // G               # 8 groups
CB = L // 128                # 4 column blocks of 128 tokens per doc

FP32 = mybir.dt.float32
FP32R = mybir.dt.float32r
INT16 = mybir.dt.int16
U_MAX = DOCS * L             # compact per-core table rows (16384 < 2**15)
IW = L // 16                 # int16 index columns per doc (32)


def build_program():
    nc = bacc.Bacc(
        "TRN2",
        target_bir_lowering=False,
        debug=False,
        num_devices=N_CORES,
    )

    table = nc.dram_tensor("table", [U_MAX, D], FP32R, kind="ExternalInput").ap()
    wt = nc.dram_tensor("wt", [D, D], FP32R, kind="ExternalInput").ap()
    idx = nc.dram_tensor("idx", [128, DOCS * (L // 16)], INT16, kind="ExternalInput").ap()
    onehot = nc.dram_tensor("onehot", [G, G * 128], FP32R, kind="ExternalInput").ap()
    out = nc.dram_tensor("out", [DOCS, D], FP32, kind="ExternalOutput").ap()

    with tile.TileContext(nc) as tc, ExitStack() as ctx:
        const = ctx.enter_context(tc.tile_pool(name="const", bufs=1))
        wtp = ctx.enter_context(tc.tile_pool(name="wtp", bufs=1))
        ep = ctx.enter_context(tc.tile_pool(name="ep", bufs=32))
        sb2 = ctx.enter_context(tc.tile_pool(name="sb2", bufs=2))
        stp = ctx.enter_context(tc.tile_pool(name="stp", bufs=4))
        spv = ctx.enter_context(tc.tile_pool(name="spv", bufs=2, space="PSUM"))
        tvp = ctx.enter_context(tc.tile_pool(name="tvp", bufs=3, space="PSUM"))
        uzp = ctx.enter_context(tc.tile_pool(name="uzp", bufs=3, space="PSUM"))

        # ---- static tiles ----
        idx_sb = const.tile([128, DOCS * IW], INT16, tag="idx")
        nc.sync.dma_start(out=idx_sb[:], in_=idx[:])

        wt_sb = []
        for k in range(8):
            t = wtp.tile([128, D], FP32R, tag=f"wt{k}", name=f"wtt{k}")
            nc.sync.dma_start(out=t[:], in_=wt[k * 128 : (k + 1) * 128, :])
            wt_sb.append(t)

        oh_sb = const.tile([G, G * 128], FP32R, tag="oh")
        nc.sync.dma_start(out=oh_sb[:], in_=onehot[:])

        ones1 = const.tile([128, 1], FP32, tag="ones1")
        nc.vector.memset(ones1[:], 1.0)
        ones1r = const.tile([128, 1], FP32R, tag="ones1r")
        nc.scalar.copy(out=ones1r[:], in_=ones1[:])
        ident = const.tile([128, 128], FP32, tag="ident")
        make_identity(nc, ident[:])

        # small leading groups start the score pipeline (DVE) early; the
        # DVE backlog that would otherwise drain after the last gather
        # shrinks by the same amount
        schedule = [(0, 1), (1, 1), (2, 2)]
        schedule += [(4 + g * G, G) for g in range(NG - 1)]
        for base, gs in schedule:
            # ---- gather the group's embedding rows (per-128-token blocks) ----
            e_tiles = []
            for j in range(gs):
                b = base + j
                quarters = []
                for qq in range(CB):
                    eq = ep.tile([128, 1, D], FP32R, tag="e", name="eq")
                    nc.gpsimd.dma_gather(
                        out_ap=eq[:],
                        in_ap=table[:],
                        idxs_ap=idx_sb[
                            :, b * IW + qq * (IW // CB) : b * IW + (qq + 1) * (IW // CB)
                        ],
                        num_idxs=L // CB,
                        num_idxs_reg=L // CB,
                        elem_size=D,
                    )
                    quarters.append(eq)
                e_tiles.append(quarters)

            # ---- s_j = sum_l emb_j as a [1, 1024] row (per doc, M=1) ----
            s_rows = []
            for j in range(gs):
                srow = sb2.tile([1, D], FP32, tag="srow", bufs=4)
                for n in range(2):
                    sp = spv.tile([1, 512], FP32, tag="spv")
                    for c in range(CB):
                        nc.tensor.matmul(
                            out=sp[0:1, :],
                            lhsT=ones1r[:],
                            rhs=e_tiles[j][c][:, 0, n * 512 : (n + 1) * 512],
                            start=(c == 0),
                            stop=(c == CB - 1),
                        )
                    nc.scalar.copy(
                        out=srow[0:1, n * 512 : (n + 1) * 512], in_=sp[0:1, :]
                    )
                s_rows.append(srow)

            # ---- transpose the 4 rows into s^T columns: T[p, j*8+k] ----
            t_ps = tvp.tile([128, G * 8], FP32, tag="tv")
            for j in range(gs):
                for k in range(8):
                    col = j * 8 + k
                    nc.tensor.transpose(
                        out=t_ps[:, col : col + 1],
                        in_=s_rows[j][0:1, k * 128 : (k + 1) * 128],
                        identity=ident[0:1, 0:1],
                    )
            st_sb = sb2.tile([128, G * 8], FP32R, tag="st_sb")
            nc.scalar.copy(out=st_sb[:, 0 : gs * 8], in_=t_ps[:, 0 : gs * 8])

            # ---- V[j, d] = sum_e s_j[e] * (W^T/L)[e, d]  (group batched) ----
            v_sb = sb2.tile([G, D], FP32R, tag="v_sb")
            for n in range(2):
                vh = spv.tile([G, 512], FP32, tag="spv", name="vh")
                for k in range(8):
                    nc.tensor.matmul(
                        out=vh[0:gs, :],
                        lhsT=st_sb[:, k : k + 8 * (gs - 1) + 1 : 8],
                        rhs=wt_sb[k][:, n * 512 : (n + 1) * 512],
                        start=(k == 0),
                        stop=(k == 7),
                    )
                nc.scalar.copy(
                    out=v_sb[0:gs, n * 512 : (n + 1) * 512], in_=vh[0:gs, :]
                )

            # ---- per-doc epilogue, streamed per 128-token block ----
            # scores, exp, and the context-sum term of block c complete as a
            # unit, so the E quarter-tile for block c is released ~4x sooner
            # than with whole-doc phases; only the final 1/Z scale waits for
            # the full softmax denominator.
            for j in range(gs):
                b = base + j
                et = e_tiles[j]
                vbs = []
                for n in range(2):
                    vb = tvp.tile([128, 512], FP32, tag="tv", name="vb")
                    nc.tensor.matmul(
                        out=vb[:],
                        lhsT=oh_sb[0:gs, j * 128 : (j + 1) * 128],
                        rhs=v_sb[0:gs, n * 512 : (n + 1) * 512],
                        start=True,
                        stop=True,
                    )
                    vbs.append(vb)
                p_sb = sb2.tile([128, CB], FP32R, tag="p_sb", bufs=4)
                zp = sb2.tile([128, CB], FP32, tag="zp", bufs=4)
                u_h = [
                    uzp.tile([1, 512], FP32, tag="uz", name=f"u{h}") for h in range(2)
                ]
                for c in range(CB):
                    sc_c = sb2.tile([128, 2], FP32, tag="sc_c", bufs=8)
                    for n in range(2):
                        scr = sb2.tile([128, 512], FP32, tag="scr")
                        nc.vector.scalar_tensor_tensor(
                            out=scr[:],
                            in0=et[c][:, 0, n * 512 : (n + 1) * 512].bitcast(FP32),
                            scalar=1.0,
                            in1=vbs[n][:],
                            op0=mybir.AluOpType.mult,
                            op1=mybir.AluOpType.mult,
                            accum_out=sc_c[:, n : n + 1],
                        )
                    scores_c = sb2.tile([128, 1], FP32, tag="scores_c", bufs=8)
                    nc.vector.tensor_tensor(
                        out=scores_c[:],
                        in0=sc_c[:, 0:1],
                        in1=sc_c[:, 1:2],
                        op=mybir.AluOpType.add,
                    )
                    nc.scalar.activation(
                        out=p_sb[:, c : c + 1],
                        in_=scores_c[:],
                        func=mybir.ActivationFunctionType.Exp,
                        accum_out=zp[:, c : c + 1],
                    )
                    for h in range(2):
                        nc.tensor.matmul(
                            out=u_h[h][0:1, :],
                            lhsT=p_sb[:, c : c + 1],
                            rhs=et[c][:, 0, h * 512 : (h + 1) * 512],
                            start=(c == 0),
                            stop=(c == CB - 1),
                        )
                zps = sb2.tile([128, 1], FP32, tag="zps", bufs=4)
                nc.vector.tensor_reduce(
                    zps[:],
                    zp[:],
                    mybir.AxisListType.X,
                    mybir.AluOpType.add,
                )
                z_ps = uzp.tile([1, 1], FP32, tag="uz", name="z_ps")
                nc.tensor.matmul(
                    out=z_ps[0:1, 0:1],
                    lhsT=ones1[:],
                    rhs=zps[:],
                    start=True,
                    stop=True,
                )
                zr = sb2.tile([1, 1], FP32, tag="zr", bufs=4)
                nc.vector.reciprocal(out=zr[:], in_=z_ps[0:1, :])
                stg = stp.tile([1, D], FP32, tag="stg", bufs=2)
                for h in range(2):
                    nc.scalar.mul(
                        out=stg[0:1, h * 512 : (h + 1) * 512],
                        in_=u_h[h][0:1, :],
                        mul=zr[0:1, 0:1],
                    )
                nc.sync.dma_start(out=out[b : b + 1, :], in_=stg[:])

    nc.compile()
    return nc


_NC = None


def _get_nc():
    global _NC
    if _NC is None:
        _NC = build_program()
    return _NC


def make_in_maps(tokens, emb_table, W_b):
    tokens = np.asarray(tokens, dtype=np.int64)
    emb_table = np.asarray(emb_table, dtype=np.float32)
    wt_np = np.ascontiguousarray(np.asarray(W_b, dtype=np.float32).T / float(L))

    onehot_np = np.zeros((G, G * 128), dtype=np.float32)
    for j in range(G):
        onehot_np[j, j * 128 : (j + 1) * 128] = 1.0

    in_maps = []
    for m in range(N_CORES):
        tok = tokens[m * DOCS : (m + 1) * DOCS]  # [32, 512]
        # compact per-core table: only the rows this core's docs reference,
        # remapped to [0, U) so indices fit the gather ucode's int16 ids
        uniq, inv = np.unique(tok + 1, return_inverse=True)
        assert uniq.size <= U_MAX
        table_np = np.zeros((U_MAX, D), dtype=np.float32)
        table_np[: uniq.size] = emb_table[uniq]
        # token order within a doc is free (sum/softmax/context-sum are
        # permutation-invariant), so sort by remapped id: the gather then
        # walks the compact table in ascending order -- near-sequential HBM
        # reads with duplicate rows adjacent, instead of random 4KB jumps
        inv16 = np.sort(inv.reshape(DOCS, L), axis=1).astype(np.int16)
        # gather ucode reads idx i from partition i%16, column i//16 of a
        # [128, L/16] tile, replicated into each 16-partition group
        blk = inv16.reshape(DOCS, IW, 16).transpose(2, 0, 1)  # [16, DOCS, IW]
        idx_np = np.ascontiguousarray(
            np.tile(blk, (8, 1, 1)).reshape(128, DOCS * IW)
        )
        in_maps.append(
            {"table": table_np, "wt": wt_np, "idx": idx_np, "onehot": onehot_np}
        )
    return in_maps


def kernel(tokens, max_len, emb_table, W_b):
    assert int(max_len) == L
    nc = _get_nc()
    in_maps = make_in_maps(tokens, emb_table, W_b)
    res = run_bass_kernel_spmd(nc, in_maps, list(range(N_CORES)))
    return np.concatenate([res.results[m]["out"] for m in range(N_CORES)], axis=0)



# revision 38
# speedup vs baseline: 1.1033x; 1.1033x over previous
"""DocumentEncoder kernel for Trainium2 (8 NeuronCores, Bass/Tile).

Reference computation (B=256, L=512, D=1024, V=50000):
    emb    = emb_table[tokens + 1]            # [B, L, D] gather
    hidden = emb.mean(axis=1)                 # [B, D]
    ha     = einsum('bld,de->ble', emb, W_b)
    scores = einsum('ble,be->bl', ha, hidden)
    attn   = softmax(scores, axis=1)
    ct     = einsum('bl,bld->bd', attn, emb)  # [B, D]

Key rewrite: scores[b,l] = emb[b,l,:] . v_b with v_b = W_b @ hidden[b] --
the [B,L,D]x[D,D] einsum collapses to [B,D]x[D,D] (~230x less compute).

Sharding: data-parallel over B (32 docs per core), table compacted per core
(unique rows, int16-indexable) and gathered per token in bf16 via the
DMAGatherAnt SWDGE ucode, 4 docs (2048 rows) per gather.

Per-core structure, built around the cost shape of the machine:
  - token-sum s^T and context-sum ct^T run as N=1 matmul floods with the
    gathered E tiles as lhsT (one 128x128 weight load per [tile, d-slice],
    output free size 1): both land in per-doc columns of persistent PSUM
    banks, d on partitions.
  - V = (W^T/L)^T s for 4-doc groups, then per-doc v broadcast across
    partitions with a K=1 ones matmul.
  - scores (the only per-token x per-d elementwise work) are split across
    engines: most 512-wide halves as fused multiply+accumulate on DVE, the
    rest as Pool multiplies reduced by ACT accumulate-copies.
  - exp on ACT with fused per-partition Z accumulation; Z finished by a
    tiny PE matmul per doc; ct columns are transposed back to doc-major
    rows on PE and scaled by 1/Z as a per-partition ACT scale.
"""

from contextlib import ExitStack

import ml_dtypes
import numpy as np

import concourse.tile as tile
from concourse import bacc, mybir
from concourse.bass_utils import run_bass_kernel_spmd
from concourse.masks import make_identity

B, L, D, V = 256, 512, 1024, 50000
N_CORES = 8
DOCS = B // N_CORES          # 32 docs per core
G = 4                        # docs per gather group
NG = DOCS // G               # 8 groups
CB = L // 128                # 4 token blocks of 128 per doc
KT = D // 128                # 8 d-slices
U_MAX = DOCS * L             # compact table rows (int16-safe)
IW = L // 16                 # int16 index columns per doc

FP32 = mybir.dt.float32
BF16 = mybir.dt.bfloat16
INT16 = mybir.dt.int16

# per-doc score routes over the 4 token blocks: even docs send blocks {2,3}
# through Pool-multiply + ACT-reduce, odd docs only block {3}; the rest run
# as fused multiply+accumulate on DVE
def pool_blocks(j):
    return ()  # bisect: all-DVE


def build_program():
    nc = bacc.Bacc(
        "TRN2",
        target_bir_lowering=False,
        debug=False,
        num_devices=N_CORES,
    )

    table = nc.dram_tensor("table", [U_MAX, D], BF16, kind="ExternalInput").ap()
    wt = nc.dram_tensor("wt", [128, KT * D], BF16, kind="ExternalInput").ap()
    onehot = nc.dram_tensor("onehot", [G, G * 128], BF16, kind="ExternalInput").ap()
    idx = nc.dram_tensor("idx", [128, DOCS * IW], INT16, kind="ExternalInput").ap()
    out = nc.dram_tensor("out", [DOCS, D], FP32, kind="ExternalOutput").ap()

    with tile.TileContext(nc) as tc, ExitStack() as ctx:
        const = ctx.enter_context(tc.tile_pool(name="const", bufs=1))
        ep = ctx.enter_context(tc.tile_pool(name="ep", bufs=3))
        sbp = ctx.enter_context(tc.tile_pool(name="sbp", bufs=1))
        scp = ctx.enter_context(tc.tile_pool(name="scp", bufs=2))
        prp = ctx.enter_context(tc.tile_pool(name="prp", bufs=3))
        pp = ctx.enter_context(tc.tile_pool(name="pp", bufs=4))
        asmp = ctx.enter_context(tc.tile_pool(name="asmp", bufs=2))
        sps = ctx.enter_context(tc.tile_pool(name="sps", bufs=1, space="PSUM"))
        vbp = ctx.enter_context(tc.tile_pool(name="vbp", bufs=2, space="PSUM"))

        # ---- static tiles ----
        idx_sb = const.tile([128, DOCS * IW], INT16, tag="idx")
        nc.sync.dma_start(out=idx_sb[:], in_=idx[:])
        wtile = const.tile([128, KT, D], BF16, tag="wt")
        nc.sync.dma_start(out=wtile[:], in_=wt[:])
        oh_sb = const.tile([G, G * 128], BF16, tag="oh")
        nc.sync.dma_start(out=oh_sb[:], in_=onehot[:])

        onesf = const.tile([128, 1], FP32, tag="onesf")
        nc.vector.memset(onesf[:], 1.0)
        onesb = const.tile([128, 1], BF16, tag="onesb")
        nc.scalar.copy(out=onesb[:], in_=onesf[:])
        ident = const.tile([128, 128], FP32, tag="ident")
        make_identity(nc, ident[:])
        identb = const.tile([128, 128], BF16, tag="identb")
        nc.scalar.copy(out=identb[:], in_=ident[:])

        # PSUM bank tiles packed by column regions
        sT_bank = sps.tile([128, KT * DOCS + KT * G], FP32, tag="sTb")
        ct_bank = sps.tile([128, KT * DOCS + DOCS], FP32, tag="ctb")  # ct^T + Z
        tr_bank = sps.tile([DOCS, 2 * 128 + 1], FP32, tag="trb")    # asm
        vr_bank = sps.tile([G, 512], FP32, tag="vrb")               # vrow asm
        sT_ps = [sT_bank[:, m * DOCS : (m + 1) * DOCS] for m in range(KT)]
        v_bank = sT_bank[:, KT * DOCS : KT * DOCS + KT * G]         # group V
        ct_ps = [ct_bank[:, m * DOCS : (m + 1) * DOCS] for m in range(KT)]
        z_ps = ct_bank[0:1, KT * DOCS : KT * DOCS + DOCS]

        for g in range(NG):
            # ---- gather 4 docs (2 gathers of 1024 rows: SWDGE ring limit) ----
            et = ep.tile([128, G * CB, D], BF16, tag="e", name="eq")
            for half in range(2):
                nc.gpsimd.dma_gather(
                    out_ap=et[:, half * 2 * CB : (half + 1) * 2 * CB, :],
                    in_ap=table[:],
                    idxs_ap=idx_sb[
                        :,
                        (g * G + half * 2) * IW : (g * G + (half + 1) * 2) * IW,
                    ],
                    num_idxs=2 * L,
                    num_idxs_reg=2 * L,
                    elem_size=D,
                )

            # ---- s^T flood: column b of sT_ps[m] = sum_tok E[tok, d] ----
            for j in range(G):
                b = g * G + j
                for m in range(KT):
                    for c in range(CB):
                        nc.tensor.matmul(
                            out=sT_ps[m][:, b : b + 1],
                            lhsT=et[:, j * CB + c, m * 128 : (m + 1) * 128],
                            rhs=onesb[:],
                            start=(c == 0),
                            stop=(c == CB - 1),
                        )
            # group s columns -> SBUF bf16
            sT_sb = asmp.tile([128, KT, G], BF16, tag="sT_sb")
            for k in range(KT):
                nc.scalar.copy(
                    out=sT_sb[:, k, :], in_=sT_ps[k][:, g * G : (g + 1) * G]
                )

            # ---- V[d, 4] = sum_e (W^T/L)[e, d] s[e, 4] for the group ----
            for m in range(KT):
                for k in range(KT):
                    nc.tensor.matmul(
                        out=v_bank[:, m * G : (m + 1) * G],
                        lhsT=wtile[:, k, m * 128 : (m + 1) * 128],
                        rhs=sT_sb[:, k, :],
                        start=(k == 0),
                        stop=(k == KT - 1),
                    )
            v_sb = asmp.tile([128, KT, G], FP32, tag="v_sb")
            for m in range(KT):
                nc.scalar.copy(
                    out=v_sb[:, m, :], in_=v_bank[:, m * G : (m + 1) * G]
                )
            # transpose to v rows [4, 1024], one 512-half at a time
            vr_sb = asmp.tile([G, D], BF16, tag="vr_sb")
            for h in range(2):
                for pos in range(4):
                    nc.tensor.transpose(
                        out=vr_bank[:, pos * 128 : (pos + 1) * 128],
                        in_=v_sb[:, 4 * h + pos, :],
                        identity=ident[:, :],
                    )
                nc.scalar.copy(
                    out=vr_sb[:, h * 512 : (h + 1) * 512], in_=vr_bank[:]
                )

            # ---- per-doc epilogue ----
            for j in range(G):
                b = g * G + j
                # broadcast v_b across partitions (one-hot group matmul)
                vb = vbp.tile([128, D], FP32, tag="vbt")
                for h in range(2):
                    nc.tensor.matmul(
                        out=vb[:, h * 512 : (h + 1) * 512],
                        lhsT=oh_sb[:, j * 128 : (j + 1) * 128],
                        rhs=vr_sb[:, h * 512 : (h + 1) * 512],
                        start=True,
                        stop=True,
                    )
                pblk = pool_blocks(j)
                vbsb = prp.tile([128, D], BF16, tag="vbsb")
                nc.scalar.copy(out=vbsb[:], in_=vb[:])
                # scores: engine-split multiply(+reduce) over token blocks
                scores = scp.tile([128, CB], FP32, tag="scores")
                for c in range(CB):
                    if c not in pblk:
                        scr = scp.tile([128, D], BF16, tag="scr")
                        nc.vector.scalar_tensor_tensor(
                            out=scr[:],
                            in0=et[:, j * CB + c, :],
                            scalar=1.0,
                            in1=vb[:],
                            op0=mybir.AluOpType.mult,
                            op1=mybir.AluOpType.mult,
                            accum_out=scores[:, c : c + 1],
                        )
                    else:
                        prod = prp.tile([128, D], BF16, tag="prod")
                        nc.gpsimd.tensor_tensor(
                            out=prod[:],
                            in0=et[:, j * CB + c, :],
                            in1=vbsb[:],
                            op=mybir.AluOpType.mult,
                        )
                        red = scp.tile([128, D], BF16, tag="red")
                        nc.scalar.activation(
                            out=red[:],
                            in_=prod[:],
                            func=mybir.ActivationFunctionType.Copy,
                            accum_out=scores[:, c : c + 1],
                        )
                # exp + per-partition Z partials, then Z via tiny PE matmul
                p_sb = pp.tile([128, CB], BF16, tag="p_sb")
                zp = pp.tile([128, 1], FP32, tag="zp")
                nc.scalar.activation(
                    out=p_sb[:],
                    in_=scores[:],
                    func=mybir.ActivationFunctionType.Exp,
                    accum_out=zp[:],
                )
                nc.tensor.matmul(
                    out=z_ps[0:1, b : b + 1],
                    lhsT=zp[:],
                    rhs=onesf[:],
                    start=True,
                    stop=True,
                )
                # ct^T flood: column b of ct_ps[m] += sum_tok p * E
                for m in range(KT):
                    for c in range(CB):
                        nc.tensor.matmul(
                            out=ct_ps[m][:, b : b + 1],
                            lhsT=et[:, j * CB + c, m * 128 : (m + 1) * 128],
                            rhs=p_sb[:, c : c + 1],
                            start=(c == 0),
                            stop=(c == CB - 1),
                        )

        # ---- normalize + transpose to doc rows, DMA out ----
        zr = sbp.tile([1, DOCS], FP32, tag="zr")
        nc.vector.reciprocal(out=zr[:], in_=z_ps)
        zc_ps = tr_bank[:, 2 * 128 : 2 * 128 + 1]
        nc.tensor.transpose(out=zc_ps, in_=zr[:], identity=ident[0:1, 0:1])
        zc = sbp.tile([DOCS, 1], FP32, tag="zc")
        nc.scalar.copy(out=zc[:], in_=zc_ps)

        out_sb = sbp.tile([DOCS, D], FP32, tag="out_sb")
        for m in range(KT):
            cts = asmp.tile([128, DOCS], FP32, tag="cts")
            nc.scalar.copy(out=cts[:], in_=ct_ps[m])
            tr = tr_bank[:, (m % 2) * 128 : (m % 2 + 1) * 128]
            nc.tensor.transpose(out=tr, in_=cts[:], identity=ident[:, :])
            nc.scalar.mul(
                out=out_sb[:, m * 128 : (m + 1) * 128], in_=tr, mul=zc[:, 0:1]
            )
        nc.sync.dma_start(out=out[:], in_=out_sb[:])

    nc.compile()
    return nc


_NC = None


def _get_nc():
    global _NC
    if _NC is None:
        _NC = build_program()
    return _NC


def make_in_maps(tokens, emb_table, W_b):
    tokens = np.asarray(tokens, dtype=np.int64)
    emb_table = np.asarray(emb_table, dtype=np.float32)
    W_b = np.asarray(W_b, dtype=np.float32)

    # wt[p, k*D + d] = W_b[d, 128k+p] / L  (lhsT tiles for v = W_b @ s/L)
    WT = np.ascontiguousarray(W_b.T) / float(L)
    wt_np = (
        WT.reshape(KT, 128, D).transpose(1, 0, 2).reshape(128, KT * D)
    ).astype(ml_dtypes.bfloat16)

    onehot_np = np.zeros((G, G * 128), dtype=ml_dtypes.bfloat16)
    for j in range(G):
        onehot_np[j, j * 128 : (j + 1) * 128] = 1.0

    in_maps = []
    for m in range(N_CORES):
        tok = tokens[m * DOCS : (m + 1) * DOCS]  # [32, 512]
        # compact per-core table: only rows this core references, remapped
        # so indices fit the gather ucode's int16 ids
        uniq, inv = np.unique(tok + 1, return_inverse=True)
        assert uniq.size <= U_MAX
        table_np = np.zeros((U_MAX, D), dtype=ml_dtypes.bfloat16)
        table_np[: uniq.size] = emb_table[uniq].astype(ml_dtypes.bfloat16)
        # sort tokens by remapped id: near-sequential HBM reads in the gather
        inv16 = np.sort(inv.reshape(DOCS, L), axis=1).astype(np.int16)
        # gather ucode reads idx i from partition i%16, column i//16,
        # replicated into each 16-partition group
        blk = inv16.reshape(DOCS, IW, 16).transpose(2, 0, 1)  # [16, DOCS, IW]
        idx_np = np.ascontiguousarray(
            np.tile(blk, (8, 1, 1)).reshape(128, DOCS * IW)
        )
        in_maps.append(
            {"table": table_np, "wt": wt_np, "idx": idx_np, "onehot": onehot_np}
        )
    return in_maps


def kernel(tokens, max_len, emb_table, W_b):
    assert int(max_len) == L
    nc = _get_nc()
    in_maps = make_in_maps(tokens, emb_table, W_b)
    res = run_bass_kernel_spmd(nc, in_maps, list(range(N_CORES)))
    return np.concatenate(
        [res.results[m]["out"] for m in range(N_CORES)], axis=0
    ).astype(np.float32)
